# revision 35
# baseline (speedup 1.0000x reference)
"""DRMGCN (dual-branch 5-layer GCN + channel attention + outer product) on
8 TRN2 NeuronCores.

Strategy (v2)
-------------
- Graph aggregation is cast as a dense matmul against the normalized
  adjacency (random graph => no usable block sparsity): agg = A_hat @ z,
  with A_hat built on host (self-loops + symmetric normalization), padded
  to 10240 nodes, stored transposed (A_hat.T, src-major) in bf16. The
  all-zero last k-chunk (rows 10112..10239) is dropped (KC=79).
- Nodes are sharded 8-way (1280/core). Each layer: local z = h @ W,
  AllGather z across cores, then aggT_shard = z_full.T-contract against
  the core's A_hat.T column slice on the tensor engine.
- Layer-0 z (= x0 @ W0) is precomputed on host and fed as a full input,
  so layer 0 starts its agg stream immediately: no warm-up AllGather.
- Emission order hides collectives: per layer, branch 1 then branch 0;
  each branch's next-layer z + AllGather is emitted right after its
  activation, so the AllGather runs under the other branch's ~100us agg
  matmul stream.
- Channel attention is per-branch (att for branch b only uses branch b's
  maxes). Conv partial products P_c = cw_c . h_c are computed per layer
  as each h_c lands (relu(att*X) == att*X since X>=0, att>0), so after
  the last layer only a tiny AllReduce(max) + 5->25->5 MLP + a DVE
  weighted sum of the P_c remain. Branch 1 finishes first: its tail and
  the oy AllGather hide under branch 0's last agg stream.
- Final [10000,128] @ [128,10000]: AllGather of the disease branch conv
  output (kept transposed [128, nodes]), each core emits a [1280, 10240]
  row-shard of the product.
- bf16 for all heavy matmul operands (fp8 adjacency was tested: e3m4
  quantization lands at 1.9e-2 rel err vs the 2e-2 gate - no margin);
  fp32 accumulation in PSUM; fp32 bias/activations; bf16 output.
"""

import numpy as np
import ml_dtypes

import concourse.mybir as mybir
import concourse.tile as tile
from concourse import bacc
from concourse.bass_utils import run_bass_kernel_spmd

NC = 8          # cores
N_NODE = 10000  # real nodes per branch
NPAD = 10240    # padded (multiple of 8*128)
S = NPAD // NC  # 1280 nodes per core
P = 128
SM = S // P     # 10 m-tiles per shard
F = 256         # feature dim
FC = F // P     # 2 feature chunks
L = 5           # gcn layers
OC = 128        # conv out channels
KC = 79         # contraction chunks (last all-zero chunk of 80 dropped)
KN = KC * P     # 10112 contraction rows actually used
NT = [(0, 512), (512, 512), (1024, 256)]  # n-tiles within a 1280 shard

F32 = mybir.dt.float32
BF16 = mybir.dt.bfloat16
BF = ml_dtypes.bfloat16
AF = mybir.ActivationFunctionType
RG = [list(range(NC))]

_CACHE = {}


def _build():
    nc = bacc.Bacc("TRN2", target_bir_lowering=False, debug=False, num_devices=NC)

    at_d, z0_d, w_d, bt_d, cwt_d, cb_d = [], [], [], [], [], []
    fc1wt_d, fc1b_d, fc2wt_d, fc2b_d = [], [], [], []
    for br in range(2):
        at_d.append(nc.dram_tensor(f"at{br}", [KC, P, S], BF16, kind="ExternalInput"))
        z0_d.append(nc.dram_tensor(f"z0{br}", [KN, F], BF16, kind="ExternalInput"))
        # W for layers 1..4 only (layer-0 z is a host-side input); weights
        # are host-transposed to partition-major so each is one contiguous DMA
        w_d.append(nc.dram_tensor(f"w{br}", [P, L - 1, FC, F], BF16, kind="ExternalInput"))
        bt_d.append(nc.dram_tensor(f"bt{br}", [P, L, FC], F32, kind="ExternalInput"))
        cwt_d.append(nc.dram_tensor(f"cwt{br}", [P, L, FC, OC], BF16, kind="ExternalInput"))
        cb_d.append(nc.dram_tensor(f"cb{br}", [P, 1], F32, kind="ExternalInput"))
        fc1wt_d.append(nc.dram_tensor(f"fc1wt{br}", [L, 5 * L], F32, kind="ExternalInput"))
        fc1b_d.append(nc.dram_tensor(f"fc1b{br}", [5 * L, 1], F32, kind="ExternalInput"))
        fc2wt_d.append(nc.dram_tensor(f"fc2wt{br}", [5 * L, L], F32, kind="ExternalInput"))
        fc2b_d.append(nc.dram_tensor(f"fc2b{br}", [L, 1], F32, kind="ExternalInput"))
    out_d = nc.dram_tensor("out", [S, NPAD], BF16, kind="ExternalOutput")

    with tile.TileContext(nc) as tc:
        with (
            tc.tile_pool(name="const", bufs=1) as const,
            tc.tile_pool(name="tl", bufs=1) as sb,
            tc.tile_pool(name="zsb", bufs=2) as zsb,
            tc.tile_pool(name="hp", bufs=2) as hp,
            tc.tile_pool(name="zk", bufs=8) as zkp,
            tc.tile_pool(name="atp", bufs=6) as atp,
            tc.tile_pool(name="ktp", bufs=3) as ktp,
            tc.tile_pool(name="fop", bufs=5) as fop,
            tc.tile_pool(name="psa", bufs=6, space="PSUM") as psa,
            tc.tile_pool(name="psz", bufs=2, space="PSUM") as psz,
            tc.tile_pool(name="dram", bufs=2, space="DRAM") as dram,
        ):
            ones_sb = const.tile([1, P], F32, name="ones_sb")
            nc.vector.memset(ones_sb[:], 1.0)

            mx_sb, w_sb, bt_sb, cwt_sb, cb_sb = [], [], [], [], []
            fc1wt_sb, fc1b_sb, fc2wt_sb, fc2b_sb = [], [], [], []
            for br in range(2):
                mx_t = const.tile([P, L], F32, name=f"mx_sb{br}")
                nc.vector.memset(mx_t[:], 0.0)
                w_t = const.tile([P, L - 1, FC, F], BF16, name=f"w_sb{br}")
                cw_t = const.tile([P, L, FC, OC], BF16, name=f"cwt_sb{br}")
                bt_t = const.tile([P, L, FC], F32, name=f"bt_sb{br}")
                cb_t = const.tile([P, 1], F32, name=f"cb_sb{br}")
                f1w = const.tile([L, 5 * L], F32, name=f"fc1wt_sb{br}")
                f1b = const.tile([5 * L, 1], F32, name=f"fc1b_sb{br}")
                f2w = const.tile([5 * L, L], F32, name=f"fc2wt_sb{br}")
                f2b = const.tile([L, 1], F32, name=f"fc2b_sb{br}")
                mx_sb.append(mx_t); w_sb.append(w_t); bt_sb.append(bt_t)
                cwt_sb.append(cw_t); cb_sb.append(cb_t)
                fc1wt_sb.append(f1w); fc1b_sb.append(f1b)
                fc2wt_sb.append(f2w); fc2b_sb.append(f2b)

            def load_consts(part):
                # emitted mid-first-agg, split in two so the burst doesn't
                # starve the k-chunk stream; everything here is first
                # consumed well after those chunks
                for br in range(2):
                    if part == 0:
                        nc.sync.dma_start(bt_sb[br][:], bt_d[br][:])
                        nc.sync.dma_start(cwt_sb[br][:], cwt_d[br][:])
                    else:
                        nc.sync.dma_start(w_sb[br][:], w_d[br][:])
                        nc.sync.dma_start(cb_sb[br][:], cb_d[br][:])
                        nc.sync.dma_start(fc1wt_sb[br][:], fc1wt_d[br][:])
                        nc.sync.dma_start(fc1b_sb[br][:], fc1b_d[br][:])
                        nc.sync.dma_start(fc2wt_sb[br][:], fc2wt_d[br][:])
                        nc.sync.dma_start(fc2b_sb[br][:], fc2b_d[br][:])

            # conv partial products P_c, accumulated per layer: [P=oc, S]
            pc_sb = [[const.tile([P, S], BF16, name=f"pc{br}_{i}") for i in range(L)]
                     for br in range(2)]
            # spare SBUF as an adjacency cache: first NCACHE k-chunks per
            # branch are loaded once (layer 0) and reused in layers 1..4
            NCACHE = 14
            atc_sb = [const.tile([P, NCACHE, S], BF16, name=f"atc{br}")
                      for br in range(2)]
            oacc_sh = const.tile([P, S], F32, name="oacc_sh")
            otmp_sh = const.tile([P, S], F32, name="otmp_sh")
            o_t = [None, None]  # attention-weighted conv outputs [P=oc, S]
            oyf_h = [None]      # gathered disease-branch conv output

            zf_cur = [z0_d[0], z0_d[1]]  # full-z source for the current layer

            def warm(n_mm):
                # junk matmuls on resident tiles: keep the PE busy through
                # act-latency / rendezvous windows so the HAM clock gate
                # doesn't re-throttle the PE to 1.2 GHz (~3.4us ramp each)
                for _ in range(n_mm):
                    jp = psz.tile([P, 512], F32, name="jp", tag="psz")
                    nc.tensor.matmul(
                        jp[:], cwt_sb[0][:, 0, 0, :], atc_sb[0][:, 0, 0:512],
                        start=True, stop=True,
                    )

            def tail_reduce(br):
                """Launch the AllReduce(max) as early as possible."""
                mxb = dram.tile([P, L], F32, name=f"mxb{br}")
                nc.sync.dma_start(mxb[:], mx_sb[br][:])
                mxr = dram.tile([P, L], F32, name=f"mxr{br}", addr_space="Shared")
                nc.gpsimd.collective_compute(
                    "AllReduce", mybir.AluOpType.max,
                    replica_groups=RG, ins=[mxb.opt()], outs=[mxr.opt()],
                )
                return mxr

            def tail_closures(br, mxr):
                """5->25->5 MLP -> att -> weighted P_c sum, as weavable steps."""
                st = {}

                def t1():
                    mrow = sb.tile([1, L, P], F32, name=f"mrow{br}")
                    nc.sync.dma_start(mrow[:], mxr.rearrange("p i -> () i p"))
                    att0 = sb.tile([1, L], F32, name=f"att0{br}")
                    nc.vector.reduce_max(att0[:], mrow[:], axis=mybir.AxisListType.X)
                    a0d = dram.tile([1, L], F32, name=f"a0d{br}")
                    nc.sync.dma_start(a0d[:], att0[:])
                    a0col = sb.tile([L, 1], F32, name=f"a0col{br}")
                    nc.sync.dma_start(a0col[:], a0d.rearrange("() c -> c ()"))
                    st["a0col"] = a0col

                def t2():
                    p1 = psz.tile([5 * L, 1], F32, name="p1", tag="psz")
                    nc.tensor.matmul(p1[:], fc1wt_sb[br][:], st["a0col"][:],
                                     start=True, stop=True)
                    y1 = sb.tile([5 * L, 1], F32, name=f"y1{br}")
                    nc.scalar.activation(y1[:], p1[:], AF.Relu, bias=fc1b_sb[br][:])
                    p2 = psz.tile([L, 1], F32, name="p2", tag="psz")
                    nc.tensor.matmul(p2[:], fc2wt_sb[br][:], y1[:], start=True, stop=True)
                    attc = sb.tile([L, 1], F32, name=f"attc{br}")
                    nc.scalar.activation(attc[:], p2[:], AF.Sigmoid, bias=fc2b_sb[br][:])
                    attf = dram.tile([1, L], F32, name=f"attf{br}")
                    nc.sync.dma_start(attf.rearrange("() c -> c ()"), attc[:])
                    attrow = sb.tile([1, L], F32, name=f"attrow{br}")
                    nc.sync.dma_start(attrow[:], attf[:])
                    st["attrow"] = attrow

                def t3():
                    pb = psz.tile([P, L], F32, name="pb", tag="psz")
                    nc.tensor.matmul(pb[:], ones_sb[:], st["attrow"][:],
                                     start=True, stop=True)
                    attb = sb.tile([P, L], F32, name=f"attb{br}")
                    nc.vector.tensor_copy(attb[:], pb[:])
                    st["attb"] = attb

                def t4():
                    # o = sum_c att_c * P_c + cb (att_c broadcast over oc)
                    acc = oacc_sh
                    nc.vector.tensor_scalar_mul(acc[:], pc_sb[br][0][:],
                                                st["attb"][:, 0:1])
                    tmp = otmp_sh
                    for c in range(1, 3):
                        nc.vector.tensor_scalar_mul(tmp[:], pc_sb[br][c][:],
                                                    st["attb"][:, c:c + 1])
                        nc.vector.tensor_tensor(acc[:], acc[:], tmp[:],
                                                mybir.AluOpType.add)
                    st["acc"] = acc
                    st["tmp"] = tmp

                def t5():
                    acc, tmp = st["acc"], st["tmp"]
                    for c in range(3, L):
                        nc.vector.tensor_scalar_mul(tmp[:], pc_sb[br][c][:],
                                                    st["attb"][:, c:c + 1])
                        nc.vector.tensor_tensor(acc[:], acc[:], tmp[:],
                                                mybir.AluOpType.add)
                    ot = const.tile([P, S], BF16, name=f"ot{br}")
                    nc.vector.tensor_scalar_add(ot[:], acc[:], cb_sb[br][:])
                    o_t[br] = ot

                steps = [t1, t2, t3, t4, t5]
                if br == 1:
                    def t6():
                        oyb = dram.tile([P, S], BF16, name="oyb")
                        nc.sync.dma_start(oyb[:], o_t[1][:])
                        oyf = dram.tile([NC * P, S], BF16, name="oyf",
                                        addr_space="Shared")
                        nc.gpsimd.collective_compute(
                            "AllGather", mybir.AluOpType.bypass,
                            replica_groups=RG, ins=[oyb.opt()], outs=[oyf.opt()],
                        )
                        oyf_h[0] = oyf
                    steps.append(t6)
                return steps

            # ---- GCN layers; branch 1 first so its tail (AllReduce + MLP +
            # oy AllGather) hides under branch 0's final agg stream.
            # The z / conv-partial / tail matmuls of each (layer, branch) are
            # WOVEN between the next agg's k-chunks: a sparse PE window
            # re-throttles the clock gate to 1.2 GHz for ~3.4us (HAM), so the
            # PE must never go sparse mid-kernel. ----
            pend_early = []  # kept for structure; closures now run inline
                             # (weaving them into the next agg measured slower:
                             # it delays the z AllGather launch by 15-40us,
                             # which costs more than the HAM warm-up it saves)

            def weave(k):
                pass

            def flush():
                pass

            def make_z(m, h_t, z_sb, br, i):
                def f():
                    zp = psz.tile([P, F], F32, name="zp", tag="psz")
                    for fc in range(FC):
                        nc.tensor.matmul(
                            zp[:], h_t[:, fc, m * P:(m + 1) * P],
                            w_sb[br][:, i, fc, :],
                            start=(fc == 0), stop=(fc == FC - 1),
                        )
                    nc.vector.tensor_copy(z_sb[:, m, :], zp[:])
                return f

            def make_zbag(z_sb, br):
                def f():
                    zb = dram.tile([S, F], BF16, name="zb")
                    nc.sync.dma_start(zb.rearrange("(m p) f -> p m f", p=P), z_sb[:])
                    zf = dram.tile([NPAD, F], BF16, name="zf", addr_space="Shared")
                    nc.gpsimd.collective_compute(
                        "AllGather", mybir.AluOpType.bypass,
                        replica_groups=RG, ins=[zb.opt()], outs=[zf.opt()],
                    )
                    zf_cur[br] = zf
                return f

            def make_pc(n, h_t, br, i):
                def f():
                    off, sz = NT[n]
                    pp = psz.tile([P, 512], F32, name="pp", tag="psz")
                    for fc in range(FC):
                        nc.tensor.matmul(
                            pp[:, :sz], cwt_sb[br][:, i, fc, :],
                            h_t[:, fc, off:off + sz],
                            start=(fc == 0), stop=(fc == FC - 1),
                        )
                    nc.vector.tensor_copy(pc_sb[br][i][:, off:off + sz], pp[:, :sz])
                return f

            for i in range(L):
                for br in (1, 0):
                    zsrc = zf_cur[br]
                    # aggT_shard = (A_hat @ z_full).T slice on this core
                    h_t = hp.tile([P, FC, S], BF16, name=f"ht{br}")
                    aps = [[psa.tile([P, 512], F32, name="aps", tag="psa")
                            for _ in NT] for _ in range(FC)]
                    for k in range(KC):
                        zk = zkp.tile([P, F], BF16, name="zk")
                        nc.sync.dma_start(zk[:], zsrc[k * P:(k + 1) * P, :])
                        if k < NCACHE:
                            if i == 0:
                                nc.sync.dma_start(atc_sb[br][:, k], at_d[br][k])
                            atk = atc_sb[br][:, k]
                        else:
                            att_t = atp.tile([P, S], BF16, name="atk")
                            nc.sync.dma_start(att_t[:], at_d[br][k])
                            atk = att_t[:]
                        for fc in range(FC):
                            for n, (off, sz) in enumerate(NT):
                                nc.tensor.matmul(
                                    aps[fc][n][:, :sz],
                                    zk[:, fc * P:(fc + 1) * P],
                                    atk[:, off:off + sz],
                                    start=(k == 0),
                                    stop=(k == KC - 1),
                                )
                        weave(k)
                        if i == 0 and br == 1 and k == 30:
                            load_consts(0)
                        if i == 0 and br == 1 and k == 48:
                            load_consts(1)
                    flush()
                    if not (i == 0 and br == 1):
                        warm(6)  # cover act latency at the boundary
                    for fc in range(FC):
                        for n, (off, sz) in enumerate(NT):
                            nc.scalar.activation(
                                h_t[:, fc, off:off + sz], aps[fc][n][:, :sz],
                                AF.Relu, bias=bt_sb[br][:, i, fc:fc + 1],
                            )
                    nc.vector.reduce_max(
                        mx_sb[br][:, i:i + 1], h_t[:], axis=mybir.AxisListType.XY,
                    )
                    # z + AllGather first (critical path for the next agg;
                    # the collective hides under the other branch's agg
                    # stream), then the conv partial
                    if i < L - 1:
                        z_sb = zsb.tile([P, SM, F], BF16, name="z_sb")
                        for m in range(SM):
                            make_z(m, h_t, z_sb, br, i)()
                        make_zbag(z_sb, br)()
                        for n in range(len(NT)):
                            make_pc(n, h_t, br, i)()
                    else:
                        mxr = tail_reduce(br)
                        for n in range(len(NT)):
                            make_pc(n, h_t, br, i)()
                        tcs = tail_closures(br, mxr)
                        tcs[0]()           # mxr readback chain
                        if br == 0:
                            warm(40)       # stay warm through the rendezvous
                        for f in tcs[1:]:
                            f()
                        if br == 0:
                            warm(24)       # stay warm into the final phase

            # ---- final: out_shard = out_x_shard @ out_y_full.T ----
            for r in range(NC):
                kt = ktp.tile([P, S], BF16, name="kt")
                nc.sync.dma_start(kt[:], oyf_h[0][r * P:(r + 1) * P, :])
                for m in range(SM):
                    fo = fop.tile([P, S], BF16, name="fo")
                    for n, (off, sz) in enumerate(NT):
                        fps = psa.tile([P, 512], F32, name="fps", tag="psa")
                        nc.tensor.matmul(
                            fps[:, :sz], o_t[0][:, m * P:(m + 1) * P],
                            kt[:, off:off + sz], start=True, stop=True,
                        )
                        # split PSUM drains across engines to keep up with PE
                        if n == 1:
                            nc.scalar.activation(fo[:, off:off + sz],
                                                 fps[:, :sz], AF.Copy)
                        else:
                            nc.vector.tensor_copy(fo[:, off:off + sz], fps[:, :sz])
                    nc.sync.dma_start(
                        out_d[m * P:(m + 1) * P, r * S:(r + 1) * S], fo[:],
                    )
    nc.compile()
    return nc


def _build_at(edges, ew):
    """Dense transposed normalized adjacency A_hat.T, padded to NPAD."""
    src = np.asarray(edges[0], dtype=np.int64)
    dst = np.asarray(edges[1], dtype=np.int64)
    w = np.asarray(ew, dtype=np.float64)
    deg = np.ones(N_NODE, dtype=np.float64)  # self loops, weight 1
    np.add.at(deg, dst, w)
    dinv = 1.0 / np.sqrt(deg)
    norm = (dinv[src] * w * dinv[dst]).astype(np.float32)
    at = np.zeros((KN, NPAD), dtype=np.float32)
    np.add.at(at, (src, dst), norm)
    ii = np.arange(N_NODE)
    at[ii, ii] += (dinv * dinv).astype(np.float32)
    return at


def _prep_branch(x, ew, W, b, cw, cb, f1w, f1b, f2w, f2b, edges):
    at = _build_at(edges, ew)
    xp = np.zeros((KN, F), dtype=np.float32)
    xp[:N_NODE] = np.asarray(x, dtype=np.float32)
    # layer-0 z precomputed on host (bf16 inputs, fp32 accum, bf16 store)
    w0 = np.asarray(W[0], np.float32).astype(BF).astype(np.float32)
    z0 = (xp.astype(BF).astype(np.float32) @ w0).astype(BF)      # [KN, F]
    wq = np.ascontiguousarray(
        np.asarray(W[1:], np.float32).reshape(L - 1, FC, P, F).transpose(2, 0, 1, 3)
    ).astype(BF)                                         # [P, L-1, FC, F]
    bt = np.ascontiguousarray(
        np.asarray(b, np.float32).reshape(L, FC, P).transpose(2, 0, 1)
    ).astype(np.float32)                                 # [P, L, FC]
    cwt = np.ascontiguousarray(
        np.asarray(cw, np.float32)[:, :, :, 0].transpose(1, 2, 0)
        .reshape(L, FC, P, OC).transpose(2, 0, 1, 3)
    ).astype(BF)                                         # [P, c, fc, oc]
    cbq = np.asarray(cb, np.float32).reshape(P, 1)
    f1wt = np.ascontiguousarray(np.asarray(f1w, np.float32).T)  # [5,25]
    f1bq = np.asarray(f1b, np.float32).reshape(5 * L, 1)
    f2wt = np.ascontiguousarray(np.asarray(f2w, np.float32).T)  # [25,5]
    f2bq = np.asarray(f2b, np.float32).reshape(L, 1)
    return at, z0, wq, bt, cwt, cbq, f1wt, f1bq, f2wt, f2bq


def _make_in_maps(inputs):
    br0 = _prep_branch(
        inputs["x_m"], inputs["w_m"], inputs["Wx"], inputs["bx"],
        inputs["cnnx_w"], inputs["cnnx_b"], inputs["fc1x_w"], inputs["fc1x_b"],
        inputs["fc2x_w"], inputs["fc2x_b"], inputs["edges_m"],
    )
    br1 = _prep_branch(
        inputs["x_d"], inputs["w_d"], inputs["Wy"], inputs["by"],
        inputs["cnny_w"], inputs["cnny_b"], inputs["fc1y_w"], inputs["fc1y_b"],
        inputs["fc2y_w"], inputs["fc2y_b"], inputs["edges_d"],
    )

    in_maps = []
    for k in range(NC):
        m = {}
        for br, (at, z0, wq, bt, cwt, cbq, f1wt, f1bq, f2wt, f2bq) in enumerate(
            (br0, br1)
        ):
            sl = slice(k * S, (k + 1) * S)
            m[f"at{br}"] = np.ascontiguousarray(at[:, sl]).astype(BF).reshape(KC, P, S)
            m[f"z0{br}"] = z0
            m[f"w{br}"] = wq
            m[f"bt{br}"] = bt
            m[f"cwt{br}"] = cwt
            m[f"cb{br}"] = cbq
            m[f"fc1wt{br}"] = f1wt
            m[f"fc1b{br}"] = f1bq
            m[f"fc2wt{br}"] = f2wt
            m[f"fc2b{br}"] = f2bq
        in_maps.append(m)
    return in_maps


def kernel(**inputs):
    if "nc" not in _CACHE:
        _CACHE["nc"] = _build()
    nc = _CACHE["nc"]
    in_maps = _make_in_maps(inputs)
    res = run_bass_kernel_spmd(nc, in_maps, core_ids=list(range(NC)))
    full = np.concatenate([res.results[k]["out"] for k in range(NC)], axis=0)
    return np.ascontiguousarray(full[:N_NODE, :N_NODE]).astype(np.float32)


# revision 36
# speedup vs baseline: 1.0218x; 1.0218x over previous
"""DRMGCN (dual-branch 5-layer GCN + channel attention + outer product) on
8 TRN2 NeuronCores.

Strategy (v2)
-------------
- Graph aggregation is cast as a dense matmul against the normalized
  adjacency (random graph => no usable block sparsity): agg = A_hat @ z,
  with A_hat built on host (self-loops + symmetric normalization), padded
  to 10240 nodes, stored transposed (A_hat.T, src-major) in bf16. The
  all-zero last k-chunk (rows 10112..10239) is dropped (KC=79).
- Nodes are sharded 8-way (1280/core). Each layer: local z = h @ W,
  AllGather z across cores, then aggT_shard = z_full.T-contract against
  the core's A_hat.T column slice on the tensor engine.
- Layer-0 z (= x0 @ W0) is precomputed on host and fed as a full input,
  so layer 0 starts its agg stream immediately: no warm-up AllGather.
- Emission order hides collectives: per layer, branch 1 then branch 0;
  each branch's next-layer z + AllGather is emitted right after its
  activation, so the AllGather runs under the other branch's ~100us agg
  matmul stream.
- Channel attention is per-branch (att for branch b only uses branch b's
  maxes). Conv partial products P_c = cw_c . h_c are computed per layer
  as each h_c lands (relu(att*X) == att*X since X>=0, att>0), so after
  the last layer only a tiny AllReduce(max) + 5->25->5 MLP + a DVE
  weighted sum of the P_c remain. Branch 1 finishes first: its tail and
  the oy AllGather hide under branch 0's last agg stream.
- Final [10000,128] @ [128,10000]: AllGather of the disease branch conv
  output (kept transposed [128, nodes]), each core emits a [1280, 10240]
  row-shard of the product.
- bf16 for all heavy matmul operands (fp8 adjacency was tested: e3m4
  quantization lands at 1.9e-2 rel err vs the 2e-2 gate - no margin);
  fp32 accumulation in PSUM; fp32 bias/activations; bf16 output.
"""

import numpy as np
import ml_dtypes

import concourse.mybir as mybir
import concourse.tile as tile
from concourse import bacc
from concourse.bass_utils import run_bass_kernel_spmd

NC = 8          # cores
N_NODE = 10000  # real nodes per branch
NPAD = 10240    # padded (multiple of 8*128)
S = NPAD // NC  # 1280 nodes per core
P = 128
SM = S // P     # 10 m-tiles per shard
F = 256         # feature dim
FC = F // P     # 2 feature chunks
L = 5           # gcn layers
OC = 128        # conv out channels
KC = 79         # contraction chunks (last all-zero chunk of 80 dropped)
KN = KC * P     # 10112 contraction rows actually used
NT = [(0, 512), (512, 512), (1024, 256)]  # n-tiles within a 1280 shard

F32 = mybir.dt.float32
BF16 = mybir.dt.bfloat16
BF = ml_dtypes.bfloat16
AF = mybir.ActivationFunctionType
RG = [list(range(NC))]

_CACHE = {}


def _build():
    nc = bacc.Bacc("TRN2", target_bir_lowering=False, debug=False, num_devices=NC)

    at_d, z0_d, w_d, bt_d, cwt_d, cb_d = [], [], [], [], [], []
    fc1wt_d, fc1b_d, fc2wt_d, fc2b_d = [], [], [], []
    for br in range(2):
        at_d.append(nc.dram_tensor(f"at{br}", [KC, P, S], BF16, kind="ExternalInput"))
        z0_d.append(nc.dram_tensor(f"z0{br}", [KN, F], BF16, kind="ExternalInput"))
        # W for layers 1..4 only (layer-0 z is a host-side input); weights
        # are host-transposed to partition-major so each is one contiguous DMA
        w_d.append(nc.dram_tensor(f"w{br}", [P, L - 1, FC, F], BF16, kind="ExternalInput"))
        bt_d.append(nc.dram_tensor(f"bt{br}", [P, L, FC], F32, kind="ExternalInput"))
        cwt_d.append(nc.dram_tensor(f"cwt{br}", [P, L, FC, OC], BF16, kind="ExternalInput"))
        cb_d.append(nc.dram_tensor(f"cb{br}", [P, 1], F32, kind="ExternalInput"))
        fc1wt_d.append(nc.dram_tensor(f"fc1wt{br}", [L, 5 * L], F32, kind="ExternalInput"))
        fc1b_d.append(nc.dram_tensor(f"fc1b{br}", [5 * L, 1], F32, kind="ExternalInput"))
        fc2wt_d.append(nc.dram_tensor(f"fc2wt{br}", [5 * L, L], F32, kind="ExternalInput"))
        fc2b_d.append(nc.dram_tensor(f"fc2b{br}", [L, 1], F32, kind="ExternalInput"))
    out_d = nc.dram_tensor("out", [S, NPAD], BF16, kind="ExternalOutput")

    with tile.TileContext(nc) as tc:
        with (
            tc.tile_pool(name="const", bufs=1) as const,
            tc.tile_pool(name="tl", bufs=1) as sb,
            tc.tile_pool(name="zsb", bufs=2) as zsb,
            tc.tile_pool(name="hp", bufs=2) as hp,
            tc.tile_pool(name="zk", bufs=8) as zkp,
            tc.tile_pool(name="atp", bufs=6) as atp,
            tc.tile_pool(name="ktp", bufs=3) as ktp,
            tc.tile_pool(name="fop", bufs=5) as fop,
            tc.tile_pool(name="psa", bufs=6, space="PSUM") as psa,
            tc.tile_pool(name="psz", bufs=2, space="PSUM") as psz,
            tc.tile_pool(name="dram", bufs=2, space="DRAM") as dram,
        ):
            ones_sb = const.tile([1, P], F32, name="ones_sb")
            nc.vector.memset(ones_sb[:], 1.0)

            mx_sb, w_sb, bt_sb, cwt_sb, cb_sb = [], [], [], [], []
            fc1wt_sb, fc1b_sb, fc2wt_sb, fc2b_sb = [], [], [], []
            for br in range(2):
                mx_t = const.tile([P, L], F32, name=f"mx_sb{br}")
                nc.vector.memset(mx_t[:], 0.0)
                w_t = const.tile([P, L - 1, FC, F], BF16, name=f"w_sb{br}")
                cw_t = const.tile([P, L, FC, OC], BF16, name=f"cwt_sb{br}")
                bt_t = const.tile([P, L, FC], F32, name=f"bt_sb{br}")
                cb_t = const.tile([P, 1], F32, name=f"cb_sb{br}")
                f1w = const.tile([L, 5 * L], F32, name=f"fc1wt_sb{br}")
                f1b = const.tile([5 * L, 1], F32, name=f"fc1b_sb{br}")
                f2w = const.tile([5 * L, L], F32, name=f"fc2wt_sb{br}")
                f2b = const.tile([L, 1], F32, name=f"fc2b_sb{br}")
                mx_sb.append(mx_t); w_sb.append(w_t); bt_sb.append(bt_t)
                cwt_sb.append(cw_t); cb_sb.append(cb_t)
                fc1wt_sb.append(f1w); fc1b_sb.append(f1b)
                fc2wt_sb.append(f2w); fc2b_sb.append(f2b)

            def load_consts(part):
                # emitted mid-first-agg, split in two so the burst doesn't
                # starve the k-chunk stream; everything here is first
                # consumed well after those chunks
                for br in range(2):
                    if part == 0:
                        nc.sync.dma_start(bt_sb[br][:], bt_d[br][:])
                        nc.sync.dma_start(cwt_sb[br][:], cwt_d[br][:])
                    else:
                        nc.sync.dma_start(w_sb[br][:], w_d[br][:])
                        nc.sync.dma_start(cb_sb[br][:], cb_d[br][:])
                        nc.sync.dma_start(fc1wt_sb[br][:], fc1wt_d[br][:])
                        nc.sync.dma_start(fc1b_sb[br][:], fc1b_d[br][:])
                        nc.sync.dma_start(fc2wt_sb[br][:], fc2wt_d[br][:])
                        nc.sync.dma_start(fc2b_sb[br][:], fc2b_d[br][:])

            # conv partial products P_c, accumulated per layer: [P=oc, S]
            pc_sb = [[const.tile([P, S], BF16, name=f"pc{br}_{i}") for i in range(L)]
                     for br in range(2)]
            # spare SBUF as an adjacency cache: first NCACHE k-chunks per
            # branch are loaded once (layer 0) and reused in layers 1..4
            NCACHE = 14
            atc_sb = [const.tile([P, NCACHE, S], BF16, name=f"atc{br}")
                      for br in range(2)]
            oacc_sh = const.tile([P, S], F32, name="oacc_sh")
            otmp_sh = const.tile([P, S], F32, name="otmp_sh")
            o_t = [None, None]  # attention-weighted conv outputs [P=oc, S]
            oyf_h = [None]      # gathered disease-branch conv output

            zf_cur = [z0_d[0], z0_d[1]]  # full-z source for the current layer

            def warm(n_mm):
                # junk matmuls on resident tiles: keep the PE busy through
                # act-latency / rendezvous windows so the HAM clock gate
                # doesn't re-throttle the PE to 1.2 GHz (~3.4us ramp each)
                for _ in range(n_mm):
                    jp = psz.tile([P, 512], F32, name="jp", tag="psz")
                    nc.tensor.matmul(
                        jp[:], cwt_sb[0][:, 0, 0, :], atc_sb[0][:, 0, 0:512],
                        start=True, stop=True,
                    )

            def tail_reduce(br):
                """Launch the AllReduce(max) as early as possible."""
                mxb = dram.tile([P, L], F32, name=f"mxb{br}")
                nc.sync.dma_start(mxb[:], mx_sb[br][:])
                mxr = dram.tile([P, L], F32, name=f"mxr{br}", addr_space="Shared")
                nc.gpsimd.collective_compute(
                    "AllReduce", mybir.AluOpType.max,
                    replica_groups=RG, ins=[mxb.opt()], outs=[mxr.opt()],
                )
                return mxr

            def tail_closures(br, mxr):
                """5->25->5 MLP -> att -> weighted P_c sum, as weavable steps."""
                st = {}

                def t1():
                    mrow = sb.tile([1, L, P], F32, name=f"mrow{br}")
                    nc.sync.dma_start(mrow[:], mxr.rearrange("p i -> () i p"))
                    att0 = sb.tile([1, L], F32, name=f"att0{br}")
                    nc.vector.reduce_max(att0[:], mrow[:], axis=mybir.AxisListType.X)
                    a0d = dram.tile([1, L], F32, name=f"a0d{br}")
                    nc.sync.dma_start(a0d[:], att0[:])
                    a0col = sb.tile([L, 1], F32, name=f"a0col{br}")
                    nc.sync.dma_start(a0col[:], a0d.rearrange("() c -> c ()"))
                    st["a0col"] = a0col

                def t2():
                    p1 = psz.tile([5 * L, 1], F32, name="p1", tag="psz")
                    nc.tensor.matmul(p1[:], fc1wt_sb[br][:], st["a0col"][:],
                                     start=True, stop=True)
                    y1 = sb.tile([5 * L, 1], F32, name=f"y1{br}")
                    nc.scalar.activation(y1[:], p1[:], AF.Relu, bias=fc1b_sb[br][:])
                    p2 = psz.tile([L, 1], F32, name="p2", tag="psz")
                    nc.tensor.matmul(p2[:], fc2wt_sb[br][:], y1[:], start=True, stop=True)
                    attc = sb.tile([L, 1], F32, name=f"attc{br}")
                    nc.scalar.activation(attc[:], p2[:], AF.Sigmoid, bias=fc2b_sb[br][:])
                    attf = dram.tile([1, L], F32, name=f"attf{br}")
                    nc.sync.dma_start(attf.rearrange("() c -> c ()"), attc[:])
                    attrow = sb.tile([1, L], F32, name=f"attrow{br}")
                    nc.sync.dma_start(attrow[:], attf[:])
                    st["attrow"] = attrow

                def t3():
                    pb = psz.tile([P, L], F32, name="pb", tag="psz")
                    nc.tensor.matmul(pb[:], ones_sb[:], st["attrow"][:],
                                     start=True, stop=True)
                    attb = sb.tile([P, L], F32, name=f"attb{br}")
                    nc.vector.tensor_copy(attb[:], pb[:])
                    st["attb"] = attb

                def t4():
                    # o = sum_c att_c * P_c + cb (att_c broadcast over oc)
                    acc = oacc_sh
                    nc.vector.tensor_scalar_mul(acc[:], pc_sb[br][0][:],
                                                st["attb"][:, 0:1])
                    tmp = otmp_sh
                    for c in range(1, 3):
                        nc.vector.tensor_scalar_mul(tmp[:], pc_sb[br][c][:],
                                                    st["attb"][:, c:c + 1])
                        nc.vector.tensor_tensor(acc[:], acc[:], tmp[:],
                                                mybir.AluOpType.add)
                    st["acc"] = acc
                    st["tmp"] = tmp

                def t5():
                    acc, tmp = st["acc"], st["tmp"]
                    for c in range(3, L):
                        nc.vector.tensor_scalar_mul(tmp[:], pc_sb[br][c][:],
                                                    st["attb"][:, c:c + 1])
                        nc.vector.tensor_tensor(acc[:], acc[:], tmp[:],
                                                mybir.AluOpType.add)
                    ot = const.tile([P, S], BF16, name=f"ot{br}")
                    nc.vector.tensor_scalar_add(ot[:], acc[:], cb_sb[br][:])
                    o_t[br] = ot

                steps = [t1, t2, t3, t4, t5]
                if br == 1:
                    def t6():
                        oyb = dram.tile([P, S], BF16, name="oyb")
                        nc.sync.dma_start(oyb[:], o_t[1][:])
                        oyf = dram.tile([NC * P, S], BF16, name="oyf",
                                        addr_space="Shared")
                        nc.gpsimd.collective_compute(
                            "AllGather", mybir.AluOpType.bypass,
                            replica_groups=RG, ins=[oyb.opt()], outs=[oyf.opt()],
                        )
                        oyf_h[0] = oyf
                    steps.append(t6)
                return steps

            # ---- GCN layers; branch 1 first so its tail (AllReduce + MLP +
            # oy AllGather) hides under branch 0's final agg stream.
            # The z / conv-partial / tail matmuls of each (layer, branch) are
            # WOVEN between the next agg's k-chunks: a sparse PE window
            # re-throttles the clock gate to 1.2 GHz for ~3.4us (HAM), so the
            # PE must never go sparse mid-kernel. ----
            pend_early = []  # kept for structure; closures now run inline
                             # (weaving them into the next agg measured slower:
                             # it delays the z AllGather launch by 15-40us,
                             # which costs more than the HAM warm-up it saves)

            def weave(k):
                pass

            def flush():
                pass

            def make_z(m, h_t, z_sb, br, i):
                def f():
                    zp = psz.tile([P, F], F32, name="zp", tag="psz")
                    for fc in range(FC):
                        nc.tensor.matmul(
                            zp[:], h_t[:, fc, m * P:(m + 1) * P],
                            w_sb[br][:, i, fc, :],
                            start=(fc == 0), stop=(fc == FC - 1),
                        )
                    nc.vector.tensor_copy(z_sb[:, m, :], zp[:])
                return f

            def make_zbag(z_sb, br):
                def f():
                    zb = dram.tile([S, F], BF16, name="zb")
                    nc.sync.dma_start(zb.rearrange("(m p) f -> p m f", p=P), z_sb[:])
                    zf = dram.tile([NPAD, F], BF16, name="zf", addr_space="Shared")
                    nc.gpsimd.collective_compute(
                        "AllGather", mybir.AluOpType.bypass,
                        replica_groups=RG, ins=[zb.opt()], outs=[zf.opt()],
                    )
                    zf_cur[br] = zf
                return f

            def make_pc(n, h_t, br, i):
                def f():
                    off, sz = NT[n]
                    pp = psz.tile([P, 512], F32, name="pp", tag="psz")
                    for fc in range(FC):
                        nc.tensor.matmul(
                            pp[:, :sz], cwt_sb[br][:, i, fc, :],
                            h_t[:, fc, off:off + sz],
                            start=(fc == 0), stop=(fc == FC - 1),
                        )
                    nc.vector.tensor_copy(pc_sb[br][i][:, off:off + sz], pp[:, :sz])
                return f

            for i in range(L):
                for br in (1, 0):
                    zsrc = zf_cur[br]
                    # aggT_shard = (A_hat @ z_full).T slice on this core
                    h_t = hp.tile([P, FC, S], BF16, name=f"ht{br}")
                    aps = [[psa.tile([P, 512], F32, name="aps", tag="psa")
                            for _ in NT] for _ in range(FC)]
                    for k in range(KC):
                        zk = zkp.tile([P, F], BF16, name="zk")
                        nc.sync.dma_start(zk[:], zsrc[k * P:(k + 1) * P, :])
                        if k < NCACHE:
                            if i == 0:
                                nc.sync.dma_start(atc_sb[br][:, k], at_d[br][k])
                            atk = atc_sb[br][:, k]
                        else:
                            att_t = atp.tile([P, S], BF16, name="atk")
                            nc.sync.dma_start(att_t[:], at_d[br][k])
                            atk = att_t[:]
                        for fc in range(FC):
                            for n, (off, sz) in enumerate(NT):
                                nc.tensor.matmul(
                                    aps[fc][n][:, :sz],
                                    zk[:, fc * P:(fc + 1) * P],
                                    atk[:, off:off + sz],
                                    start=(k == 0),
                                    stop=(k == KC - 1),
                                )
                        weave(k)
                        if i == 0 and br == 1 and k == 30:
                            load_consts(0)
                        if i == 0 and br == 1 and k == 48:
                            load_consts(1)
                    flush()
                    for fc in range(FC):
                        for n, (off, sz) in enumerate(NT):
                            nc.scalar.activation(
                                h_t[:, fc, off:off + sz], aps[fc][n][:, :sz],
                                AF.Relu, bias=bt_sb[br][:, i, fc:fc + 1],
                            )
                    nc.vector.reduce_max(
                        mx_sb[br][:, i:i + 1], h_t[:], axis=mybir.AxisListType.XY,
                    )
                    # z + AllGather first (critical path for the next agg;
                    # the collective hides under the other branch's agg
                    # stream), then the conv partial
                    if i < L - 1:
                        z_sb = zsb.tile([P, SM, F], BF16, name="z_sb")
                        for m in range(SM):
                            make_z(m, h_t, z_sb, br, i)()
                        make_zbag(z_sb, br)()
                        for n in range(len(NT)):
                            make_pc(n, h_t, br, i)()
                    else:
                        mxr = tail_reduce(br)
                        for n in range(len(NT)):
                            make_pc(n, h_t, br, i)()
                        for f in tail_closures(br, mxr):
                            f()

            # ---- final: out_shard = out_x_shard @ out_y_full.T ----
            for r in range(NC):
                kt = ktp.tile([P, S], BF16, name="kt")
                nc.sync.dma_start(kt[:], oyf_h[0][r * P:(r + 1) * P, :])
                for m in range(SM):
                    fo = fop.tile([P, S], BF16, name="fo")
                    for n, (off, sz) in enumerate(NT):
                        fps = psa.tile([P, 512], F32, name="fps", tag="psa")
                        nc.tensor.matmul(
                            fps[:, :sz], o_t[0][:, m * P:(m + 1) * P],
                            kt[:, off:off + sz], start=True, stop=True,
                        )
                        # split PSUM drains across engines to keep up with PE
                        if n == 1:
                            nc.scalar.activation(fo[:, off:off + sz],
                                                 fps[:, :sz], AF.Copy)
                        else:
                            nc.vector.tensor_copy(fo[:, off:off + sz], fps[:, :sz])
                    nc.sync.dma_start(
                        out_d[m * P:(m + 1) * P, r * S:(r + 1) * S], fo[:],
                    )
    nc.compile()
    return nc


def _build_at(edges, ew):
    """Dense transposed normalized adjacency A_hat.T, padded to NPAD."""
    src = np.asarray(edges[0], dtype=np.int64)
    dst = np.asarray(edges[1], dtype=np.int64)
    w = np.asarray(ew, dtype=np.float64)
    deg = np.ones(N_NODE, dtype=np.float64)  # self loops, weight 1
    np.add.at(deg, dst, w)
    dinv = 1.0 / np.sqrt(deg)
    norm = (dinv[src] * w * dinv[dst]).astype(np.float32)
    at = np.zeros((KN, NPAD), dtype=np.float32)
    np.add.at(at, (src, dst), norm)
    ii = np.arange(N_NODE)
    at[ii, ii] += (dinv * dinv).astype(np.float32)
    return at


def _prep_branch(x, ew, W, b, cw, cb, f1w, f1b, f2w, f2b, edges):
    at = _build_at(edges, ew)
    xp = np.zeros((KN, F), dtype=np.float32)
    xp[:N_NODE] = np.asarray(x, dtype=np.float32)
    # layer-0 z precomputed on host (bf16 inputs, fp32 accum, bf16 store)
    w0 = np.asarray(W[0], np.float32).astype(BF).astype(np.float32)
    z0 = (xp.astype(BF).astype(np.float32) @ w0).astype(BF)      # [KN, F]
    wq = np.ascontiguousarray(
        np.asarray(W[1:], np.float32).reshape(L - 1, FC, P, F).transpose(2, 0, 1, 3)
    ).astype(BF)                                         # [P, L-1, FC, F]
    bt = np.ascontiguousarray(
        np.asarray(b, np.float32).reshape(L, FC, P).transpose(2, 0, 1)
    ).astype(np.float32)                                 # [P, L, FC]
    cwt = np.ascontiguousarray(
        np.asarray(cw, np.float32)[:, :, :, 0].transpose(1, 2, 0)
        .reshape(L, FC, P, OC).transpose(2, 0, 1, 3)
    ).astype(BF)                                         # [P, c, fc, oc]
    cbq = np.asarray(cb, np.float32).reshape(P, 1)
    f1wt = np.ascontiguousarray(np.asarray(f1w, np.float32).T)  # [5,25]
    f1bq = np.asarray(f1b, np.float32).reshape(5 * L, 1)
    f2wt = np.ascontiguousarray(np.asarray(f2w, np.float32).T)  # [25,5]
    f2bq = np.asarray(f2b, np.float32).reshape(L, 1)
    return at, z0, wq, bt, cwt, cbq, f1wt, f1bq, f2wt, f2bq


def _make_in_maps(inputs):
    br0 = _prep_branch(
        inputs["x_m"], inputs["w_m"], inputs["Wx"], inputs["bx"],
        inputs["cnnx_w"], inputs["cnnx_b"], inputs["fc1x_w"], inputs["fc1x_b"],
        inputs["fc2x_w"], inputs["fc2x_b"], inputs["edges_m"],
    )
    br1 = _prep_branch(
        inputs["x_d"], inputs["w_d"], inputs["Wy"], inputs["by"],
        inputs["cnny_w"], inputs["cnny_b"], inputs["fc1y_w"], inputs["fc1y_b"],
        inputs["fc2y_w"], inputs["fc2y_b"], inputs["edges_d"],
    )

    in_maps = []
    for k in range(NC):
        m = {}
        for br, (at, z0, wq, bt, cwt, cbq, f1wt, f1bq, f2wt, f2bq) in enumerate(
            (br0, br1)
        ):
            sl = slice(k * S, (k + 1) * S)
            m[f"at{br}"] = np.ascontiguousarray(at[:, sl]).astype(BF).reshape(KC, P, S)
            m[f"z0{br}"] = z0
            m[f"w{br}"] = wq
            m[f"bt{br}"] = bt
            m[f"cwt{br}"] = cwt
            m[f"cb{br}"] = cbq
            m[f"fc1wt{br}"] = f1wt
            m[f"fc1b{br}"] = f1bq
            m[f"fc2wt{br}"] = f2wt
            m[f"fc2b{br}"] = f2bq
        in_maps.append(m)
    return in_maps


def kernel(**inputs):
    if "nc" not in _CACHE:
        _CACHE["nc"] = _build()
    nc = _CACHE["nc"]
    in_maps = _make_in_maps(inputs)
    res = run_bass_kernel_spmd(nc, in_maps, core_ids=list(range(NC)))
    full = np.concatenate([res.results[k]["out"] for k in range(NC)], axis=0)
    return np.ascontiguousarray(full[:N_NODE, :N_NODE]).astype(np.float32)
